# revision 58
# baseline (speedup 1.0000x reference)
"""Multi-head attention (B=4, N=2048, DM=1024, H=16, DH=64) on 8 trn2 cores.

Sharding: core c -> (batch b = c//2, head-group hg = c%2 of 8 heads).

Dual token compaction: only tokens with mask=1 participate in attention --
as queries AND as keys/values.  The host gathers the ~1024 live tokens per
batch into a fixed NL=1152-slot compact array (zero padded).  Padded slots
have x=0, so k=0 (scores 0 -> exp 1) and v=0, and their vplus "ones"
column entry is 0 (live-indicator input), so they contribute nothing to
either the attention numerator or denominator.  Live queries then see
exactly the reference's masked softmax (exp(-1e6)=0 for dead keys).

Dead-query rows are identical within a batch (softmax over an all-equal
row is uniform over ALL tokens): out_dead = mean_j(v_j) @ w_out + b_out,
computed exactly in f32 on host and scattered into the output.

Device-side layout ("feature-major"), per core:
  - xcT [DM, NL]: compact tokens, feature-major.
  - QK projection emits q^T/k^T [64, NL] per head directly (SCALE baked
    into w_q); scores s^T [j, i] tiles feed exp on ACT; exp'd tiles t
    feed the PV matmul where an appended per-head ones column (the live
    indicator) accumulates the softmax denominator for free.
  - per-head-pair QK projection, V projection and the output projection
    are interleaved into the softmax head loop so the PE keeps busy while
    ACT (exp) drains.
"""

import sys

sys.path.insert(0, "/opt/trn_rl_repo")

import numpy as np
import ml_dtypes

B, N, DM, H, DH = 4, 2048, 1024, 16, 64
SCALE = DH**-0.5
NCORES = 8
HG = 2  # head groups (tensor-parallel factor)
HL = H // HG  # 8 heads per core
NP = HL // 2  # 4 f-tile pairs (2 heads per 128-partition f-tile)
P = 128
DMT = DM // P  # 8 dm tiles
FQK = HL * 2 * DH  # 1024 qk features per core
FV = HL * DH  # 512 v features per core
HT = FV // P  # 4 head-dim tiles for the projection
VW = DH + 1  # 65: v columns + ones column
VROW = HL * VW  # 520
NTL = 9  # compact token tiles (capacity NTL*128 = 1152 live tokens)

_CACHE = {}


def _build_program(ntl):
    import concourse.mybir as mybir
    import concourse.tile as tile
    from concourse import bacc
    from concourse.bass import ts
    from concourse.masks import make_identity

    bf = mybir.dt.bfloat16
    f32 = mybir.dt.float32
    EXP = mybir.ActivationFunctionType.Exp
    COPY = mybir.ActivationFunctionType.Copy

    NL = ntl * P
    # moving-dim chunks (matmul moving free dim <= 512)
    chunks = [(c, min(512, NL - c)) for c in range(0, NL, 512)]
    HALF0 = (ntl + 1) // 2  # PV i-tile split (5/4 for ntl=9)
    NA = 1024  # scores i-split: A tile [P,1024] (2 banks) + B tile (rest)
    NB = NL - NA

    nc = bacc.Bacc(
        "TRN2", target_bir_lowering=False, debug=False, num_devices=NCORES
    )
    xcT = nc.dram_tensor("xcT", [DM, NL], bf, kind="ExternalInput")
    wqk = nc.dram_tensor("wqk", [DM, FQK], bf, kind="ExternalInput")
    wv = nc.dram_tensor("wv", [DM, FV], bf, kind="ExternalInput")
    wout = nc.dram_tensor("wout", [FV, DM], bf, kind="ExternalInput")
    onesc = nc.dram_tensor("onesc", [P, ntl], f32, kind="ExternalInput")
    out = nc.dram_tensor("out", [NL, DM], f32, kind="ExternalOutput")

    with tile.TileContext(nc) as tc:
        with tc.tile_pool(name="const", bufs=1) as cp:
            xcT_sb = cp.tile([P, DMT * NL], bf, tag="xcT")
            wqk_sb = cp.tile([P, DMT * FQK], bf, tag="wqk")
            wv_sb = cp.tile([P, DMT * FV], bf, tag="wv")
            wout_sb = cp.tile([P, HT * DM], bf, tag="wout")
            onesc_sb = cp.tile([P, ntl], f32, tag="onesc")
            ones8 = cp.tile([P, HL], bf, tag="ones8")
            ident = cp.tile([P, P], bf, tag="ident")
            vplus = cp.tile([P, ntl * VROW], bf, tag="vplus")
            qk_all = cp.tile([P, (FQK // P) * NL], bf, tag="qkall")
            attT = cp.tile([P, HT * NL], bf, tag="attT")
            # bf16 partial: head pairs 0+1 contribution to out cols 0:512
            o_part = cp.tile([P, ntl * 512], bf, tag="opart")
            # one att buffer per head pair so transposes can be deferred
            # into the filler-less late rounds
            att_pair = [
                cp.tile([P, ntl * P], bf, tag=f"attpair{p}", name=f"att_pair{p}")
                for p in range(NP)
            ]

            # DMA order mirrors consumption.  wqk is pair-blocked on host
            # (columns [q_p0|k_p0|q_p1|k_p1|...]) so each head pair is one
            # merged all-dmt DMA; pair 0 leads, then per-dmt xcT (so the
            # first projection's accumulation can start on dmt 0 while
            # later dmt tiles are still in flight), then wv / remaining
            # pairs / wout.  Few, large DMAs: each dma_start costs ~625ns
            # of serialized HWDGE issue time.
            wqk_sb3 = wqk_sb.rearrange("p (d f) -> p d f", d=DMT, f=FQK)
            wqk3 = wqk.rearrange("(d p) f -> p d f", d=DMT, p=P)
            nc.sync.dma_start(
                out=wqk_sb3[:, :, 0 : 2 * P], in_=wqk3[:, :, 0 : 2 * P]
            )
            for dmt in range(DMT):
                nc.sync.dma_start(out=xcT_sb[:, ts(dmt, NL)], in_=xcT[ts(dmt, P), :])
            nc.sync.dma_start(out=onesc_sb[:, :], in_=onesc[:, :])
            nc.sync.dma_start(
                out=wv_sb.rearrange("p (d f) -> p d f", d=DMT, f=FV),
                in_=wv.rearrange("(d p) f -> p d f", d=DMT, p=P),
            )
            for pair in range(1, NP):
                nc.sync.dma_start(
                    out=wqk_sb3[:, :, 2 * pair * P : 2 * (pair + 1) * P],
                    in_=wqk3[:, :, 2 * pair * P : 2 * (pair + 1) * P],
                )
            nc.sync.dma_start(
                out=wout_sb.rearrange("p (t f) -> p t f", t=HT, f=DM),
                in_=wout.rearrange("(t p) f -> p t f", t=HT, p=P),
            )
            make_identity(nc, ident)
            nc.gpsimd.memset(ones8, 1.0)
            vp4 = vplus.rearrange("p (t g c) -> p t g c", t=ntl, g=HL, c=VW)

            # Prime the DVE vector clock on the onesc DMA so the first
            # real DVE op needs only its PE wait.
            scratch = cp.tile([1, 1], f32, tag="scratch")
            nc.vector.tensor_copy(scratch, onesc_sb[0:1, 0:1])

            with (
                tc.tile_pool(name="pA", bufs=2, space="PSUM") as pA,
                tc.tile_pool(name="pmisc", bufs=2, space="PSUM") as pmisc,
                tc.tile_pool(name="ppv", bufs=2, space="PSUM") as ppv,
                tc.tile_pool(name="tpool", bufs=38) as tp,
                tc.tile_pool(name="spool", bufs=6) as sp,
            ):

                def emit_proj_chunk(ft, c0, cw):
                    ps = pmisc.tile([P, cw], f32, tag="m", name="ps_p")
                    for dmt in range(DMT):
                        nc.tensor.matmul(
                            ps[:, :],
                            wqk_sb[:, dmt * FQK + ft * P : dmt * FQK + (ft + 1) * P],
                            xcT_sb[:, dmt * NL + c0 : dmt * NL + c0 + cw],
                            start=(dmt == 0),
                            stop=(dmt == DMT - 1),
                        )
                    nc.vector.tensor_copy(
                        qk_all[:, ft * NL + c0 : ft * NL + c0 + cw], ps[:, :]
                    )

                def emit_vproj(tt):
                    ps = pmisc.tile([P, FV], f32, tag="m", name="ps_v")
                    for dmt in range(DMT):
                        nc.tensor.matmul(
                            ps[:, :],
                            xcT_sb[:, dmt * NL + tt * P : dmt * NL + (tt + 1) * P],
                            wv_sb[:, ts(dmt, FV)],
                            start=(dmt == 0),
                            stop=(dmt == DMT - 1),
                        )
                    nc.vector.tensor_copy(
                        vp4[:, tt, :, 0:DH],
                        ps.rearrange("p (g c) -> p g c", g=HL, c=DH),
                    )
                    # live-indicator -> per-head ones column
                    nc.vector.tensor_scalar_mul(
                        vp4[:, tt, :, DH], ones8[:, :], onesc_sb[:, tt : tt + 1]
                    )

                # grouped-tail state for head 6 (emitted in round 3, the
                # only round with no other pmisc traffic)
                g6 = {}

                def emit_scores(h, jt):
                    # scores split into an A tile (i 0:1024) and a small B
                    # tile (i 1024:NL): three 2-bank slots instead of two
                    # 3-bank ones, so exp's slot-release chain stops
                    # metering the pipeline down to ~75% ACT duty.
                    pair, hh = h // 2, h % 2
                    p0 = hh * DH
                    kcol = (2 * pair + 1) * NL
                    qcol = 2 * pair * NL
                    kT = qk_all[p0 : p0 + DH, kcol + jt * P : kcol + (jt + 1) * P]
                    t_sb = tp.tile([P, NL], bf, tag="t", name="t_sb")
                    ps_a = pA.tile([P, NA], f32, tag="a", name="ps_a")
                    for c0 in range(0, NA, 512):
                        nc.tensor.matmul(
                            ps_a[:, c0 : c0 + 512],
                            kT,
                            qk_all[p0 : p0 + DH, qcol + c0 : qcol + c0 + 512],
                            start=True,
                            stop=True,
                        )
                    nc.scalar.activation(t_sb[:, 0:NA], ps_a[:, :], EXP)
                    if h == 6:
                        # batch three j-tiles' tail scores into one PSUM
                        # tile and one exp (into a consolidated per-head
                        # tail tensor), amortizing ACT's fixed per-inst
                        # access latency; safe only in a round whose misc
                        # pool has no other users.
                        g = jt % 3
                        if jt == 0:
                            g6["tt"] = tp.tile([P, NL], bf, tag="t", name="t_tail6")
                        if g == 0:
                            g6["ps"] = pmisc.tile(
                                [P, 3 * NB], f32, tag="m", name="ps_bg"
                            )
                            g6["j0"] = jt
                        nc.tensor.matmul(
                            g6["ps"][:, g * NB : (g + 1) * NB],
                            kT,
                            qk_all[p0 : p0 + DH, qcol + NA : qcol + NL],
                            start=(g == 0),
                            stop=(g == 2 or jt == ntl - 1),
                        )
                        if g == 2 or jt == ntl - 1:
                            n = jt - g6["j0"] + 1
                            nc.scalar.activation(
                                g6["tt"][:, g6["j0"] * NB : (jt + 1) * NB],
                                g6["ps"][:, 0 : n * NB],
                                EXP,
                            )
                        return t_sb
                    ps_b = pmisc.tile([P, NB], f32, tag="m", name="ps_b")
                    nc.tensor.matmul(
                        ps_b[:, :],
                        kT,
                        qk_all[p0 : p0 + DH, qcol + NA : qcol + NL],
                        start=True,
                        stop=True,
                    )
                    nc.scalar.activation(t_sb[:, NA:NL], ps_b[:, :], EXP)
                    return t_sb

                def emit_pv(h, t_tiles, cb=None, t_tail=None):
                    # PV output packed at stride VW=65 so a 5-tile half
                    # fits one PSUM bank; the two halves run in the two
                    # ppv slots concurrently, interleaved per j-tile so
                    # PE work is paced evenly against ACT's exp drain.
                    nit1 = ntl - HALF0
                    p0 = (h % 2) * DH
                    pa0 = ppv.tile([P, HALF0 * VW], f32, tag="pa", name="pa0")
                    pa1 = ppv.tile([P, HALF0 * VW], f32, tag="pa", name="pa1")
                    for jt in range(ntl):
                        if cb is not None:
                            cb(jt)
                        vs = vplus[:, jt * VROW + h * VW : jt * VROW + (h + 1) * VW]
                        for i in range(HALF0):
                            nc.tensor.matmul(
                                pa0[:, i * VW : (i + 1) * VW],
                                t_tiles[jt][:, ts(i, P)],
                                vs,
                                start=(jt == 0 and i == 0),
                                stop=(jt == ntl - 1 and i == HALF0 - 1),
                            )
                        for i in range(nit1):
                            if t_tail is not None and HALF0 + i == ntl - 1:
                                src = t_tail[:, ts(jt, P)]
                            else:
                                src = t_tiles[jt][:, ts(HALF0 + i, P)]
                            nc.tensor.matmul(
                                pa1[:, i * VW : (i + 1) * VW],
                                src,
                                vs,
                                start=(jt == 0 and i == 0),
                                stop=(jt == ntl - 1 and i == nit1 - 1),
                            )
                    ap = att_pair[h // 2]
                    for half, pa, i0, nit in (
                        (0, pa0, 0, HALF0),
                        (1, pa1, HALF0, nit1),
                    ):
                        pa3 = pa.rearrange("p (t c) -> p t c", t=HALF0, c=VW)
                        r_sb = sp.tile([P, nit], f32, tag="r", name="r_sb")
                        nc.vector.reciprocal(r_sb[:, :], pa3[:, 0:nit, DH])
                        for i in range(nit):
                            it = i0 + i
                            nc.vector.tensor_scalar_mul(
                                ap[:, it * P + p0 : it * P + p0 + DH],
                                pa[:, i * VW : i * VW + DH],
                                r_sb[:, i : i + 1],
                            )

                def emit_transposes(pair):
                    for it in range(ntl):
                        ps_tr = pmisc.tile([P, P], bf, tag="m", name="ps_tr")
                        nc.tensor.transpose(
                            ps_tr[:, :], att_pair[pair][:, ts(it, P)], ident
                        )
                        nc.vector.tensor_copy(
                            attT[:, pair * NL + it * P : pair * NL + (it + 1) * P],
                            ps_tr[:, :],
                        )

                def emit_partial(it):
                    # head pairs 0+1 contribution to output columns 0:512,
                    # computed in round 4's PE idle (during the exp
                    # stream) and stored bf16; folded into the final
                    # chunk via an identity matmul.
                    ps = pmisc.tile([P, 512], f32, tag="m", name="ps_po")
                    for ht in range(2):
                        nc.tensor.matmul(
                            ps[:, :],
                            attT[:, ht * NL + it * P : ht * NL + (it + 1) * P],
                            wout_sb[:, ht * DM : ht * DM + 512],
                            start=(ht == 0),
                            stop=(ht == 1),
                        )
                    nc.vector.tensor_copy(
                        o_part[:, it * 512 : (it + 1) * 512], ps[:, :]
                    )

                def emit_outproj(it, ch):
                    # alternate pools: pA is idle by now, so this gives 4
                    # slots and unthrottles the chunk pipeline
                    pool = pA if (2 * it + ch) % 2 else pmisc
                    tag = "a" if pool is pA else "m"
                    ps_o = pool.tile([P, 512], f32, tag=tag, name="ps_o")
                    hts = (2, 3) if ch == 0 else (0, 1, 2, 3)
                    for ht in hts:
                        nc.tensor.matmul(
                            ps_o[:, :],
                            attT[:, ht * NL + it * P : ht * NL + (it + 1) * P],
                            wout_sb[:, ht * DM + ch * 512 : ht * DM + (ch + 1) * 512],
                            start=(ht == hts[0]),
                            stop=(ch == 1 and ht == hts[-1]),
                        )
                    if ch == 0:
                        nc.tensor.matmul(
                            ps_o[:, :],
                            ident[:, :],
                            o_part[:, it * 512 : (it + 1) * 512],
                            start=False,
                            stop=True,
                        )
                    o_sb = sp.tile([P, 512], f32, tag="o", name="o_sb")
                    # Alternate eviction engine so ACT and DVE each drain
                    # half the projection chunks in parallel.
                    if ch == 0:
                        nc.scalar.activation(o_sb[:, :], ps_o[:, :], COPY)
                    else:
                        nc.vector.tensor_copy(o_sb[:, :], ps_o[:, :])
                    nc.sync.dma_start(out=out[ts(it, P), ts(ch, 512)], in_=o_sb[:, :])

                # PE p-state warm-up: ~2.8us of dummy transposes during
                # the initial DMA wait so the first projections run at
                # full clock (the cost model ramps 0.65->2.4GHz over 3us
                # of continuous execution).
                for _ in range(14):
                    ps_w = pmisc.tile([P, P], bf, tag="m", name="ps_w")
                    nc.tensor.transpose(ps_w[:, :], ident[:, :], ident[:, :])

                # ---- prologue: pair-0 projection, heads 0-1 scores,
                # V projection and pair-1 projection.  ACT would idle here
                # anyway (nothing to exp before the first scores), so this
                # builds a two-head exp backlog that keeps ACT saturated
                # through the rounds.  The k projection is chunked so the
                # first scores tiles (which only need k columns jt*128..)
                # start as soon as possible.
                for c0, cw in chunks:
                    emit_proj_chunk(0, c0, cw)
                t_by_h = {0: [], 1: []}
                kchunks = iter(chunks)
                p1 = [(ft, c0, cw) for ft in (2, 3) for (c0, cw) in chunks]
                p1i = iter(range(len(p1)))
                for k in range(2 * ntl):
                    h01, jt = divmod(k, ntl)
                    if h01 == 0 and jt * P % 512 == 0:
                        nxt_k = next(kchunks, None)
                        if nxt_k is not None:
                            emit_proj_chunk(1, *nxt_k)
                    t_by_h[h01].append(emit_scores(h01, jt))
                    if k % 2 == 1:
                        emit_vproj(k // 2)
                    if k >= 10:
                        i = next(p1i, None)
                        if i is not None:
                            emit_proj_chunk(*p1[i])
                # third lookahead head: more exp backlog (ACT is the
                # binding engine from here on)
                t_by_h[2] = [emit_scores(2, jt) for jt in range(ntl)]

                # Pair 2/3 projections become 128-wide filler granules
                # spread over rounds 0-2 (pair p's granules must all
                # precede the first scores(2p): PE executes in emission
                # order, and scores(2p) now appears in round 2p-3).
                filler = [
                    (ft, c0, P)
                    for pair2 in range(2, NP)
                    for ft in (2 * pair2, 2 * pair2 + 1)
                    for c0 in range(0, NL, P)
                ]
                g = 2 * ntl  # granules per f-tile pair
                plan = {0: filler[:g], 1: filler[g : g + ntl],
                        2: filler[g + ntl :]}

                # Transposes of pair p are deferred into the filler-less
                # late rounds to keep the PE fed there.
                tr_plan = {2: (0,), 3: (1,), 6: (2,), 7: (3,)}

                # ---- head rounds: scores(h+3) x PV(h) pipeline ----
                # The LAST head's scores are trickled through rounds 4-6
                # instead of all landing in round 4: their PSUM allocs
                # gate on the exp drain, and emitting them all at once
                # would block rounds 5-6's PV/transpose work behind them
                # (PE executes in emission order), pushing it past the
                # end of the exp stream.
                last = HL - 1
                trickle = {
                    4: {jt: jt for jt in range(ntl - 4)},
                    5: {1: ntl - 4, 5: ntl - 3},
                    6: {1: ntl - 2, 5: ntl - 1},
                }
                for h in range(HL):
                    fill_h = plan.get(h, [])
                    emitted = [0]
                    if h + 3 < last:
                        t_by_h[h + 3] = []
                    if h == 4:
                        t_by_h[last] = []

                    def cb(jt, h=h, fill_h=fill_h, emitted=emitted):
                        if h + 3 < last:
                            t_by_h[h + 3].append(emit_scores(h + 3, jt))
                        tk = trickle.get(h, {}).get(jt)
                        if tk is not None:
                            t_by_h[last].append(emit_scores(last, tk))
                        if h == 4:
                            emit_partial(jt)
                        # even spread of this round's granules over 9 slots
                        quota = (len(fill_h) * (jt + 1) + ntl - 1) // ntl
                        while emitted[0] < quota:
                            emit_proj_chunk(*fill_h[emitted[0]])
                            emitted[0] += 1

                    emit_pv(
                        h,
                        t_by_h[h],
                        cb,
                        t_tail=g6.get("tt") if h == 6 else None,
                    )
                    del t_by_h[h]
                    for pair_tr in tr_plan.get(h, ()):
                        if pair_tr < NP - 1:
                            emit_transposes(pair_tr)

                # ---- output projection, interleaved with the last
                # pair's transposes (2 i-tiles of lookahead so the
                # transpose->copy chain stays off the chunk pipeline) ----
                def emit_tr3(it):
                    ps_tr = pmisc.tile([P, P], bf, tag="m", name="ps_tr")
                    nc.tensor.transpose(
                        ps_tr[:, :], att_pair[NP - 1][:, ts(it, P)], ident
                    )
                    nc.vector.tensor_copy(
                        attT[:, (NP - 1) * NL + it * P : (NP - 1) * NL + (it + 1) * P],
                        ps_tr[:, :],
                    )

                emit_tr3(0)
                emit_tr3(1)
                for it in range(ntl):
                    if it + 2 < ntl:
                        emit_tr3(it + 2)
                    for ch in range(2):
                        emit_outproj(it, ch)

    nc.compile()
    return nc


def _host_prep(x, w_qkv, w_out, b_out, mask):
    """Per-core device inputs (compaction + layout) and live-index meta."""
    bf = ml_dtypes.bfloat16
    x = np.asarray(x, dtype=np.float32)
    w_qkv = np.asarray(w_qkv, dtype=np.float32)
    w_out = np.asarray(w_out, dtype=np.float32)
    mask = np.asarray(mask)

    w3 = w_qkv.reshape(DM, H, 3, DH)
    idxs = [np.nonzero(mask[b])[0] for b in range(B)]
    ntl = max(NTL, -(-max(len(i) for i in idxs) // P))
    NL = ntl * P
    in_maps = []
    for c in range(NCORES):
        b, hg = c // HG, c % HG
        idx = idxs[b]
        nl = len(idx)
        xc = np.zeros((NL, DM), np.float32)
        xc[:nl] = x[b][idx]
        xcT_c = np.ascontiguousarray(xc.T).astype(bf)
        # pair-blocked qk features: [q_p0 | k_p0 | q_p1 | k_p1 | ...],
        # 128 cols each (2 heads x 64); q pre-scaled by SCALE.
        blocks = []
        for p in range(NP):
            h0 = hg * HL + 2 * p
            blocks.append(w3[:, h0 : h0 + 2, 0, :].reshape(DM, 2 * DH) * SCALE)
            blocks.append(w3[:, h0 : h0 + 2, 1, :].reshape(DM, 2 * DH))
        wqk_c = np.ascontiguousarray(np.concatenate(blocks, axis=1)).astype(bf)
        wv_c = np.ascontiguousarray(
            w3[:, hg * HL : (hg + 1) * HL, 2, :].reshape(DM, FV)
        ).astype(bf)
        wout_c = np.ascontiguousarray(w_out[hg * FV : (hg + 1) * FV, :]).astype(bf)
        ind = np.zeros((NL,), np.float32)
        ind[:nl] = 1.0
        onesc_c = np.ascontiguousarray(ind.reshape(ntl, P).T)
        in_maps.append(
            {
                "xcT": xcT_c,
                "wqk": wqk_c,
                "wv": wv_c,
                "wout": wout_c,
                "onesc": onesc_c,
            }
        )
    return in_maps, idxs, ntl


def _shard_inputs(x, w_qkv, w_out, b_out, mask):
    return _host_prep(x, w_qkv, w_out, b_out, mask)[0]


def _assemble(parts, x, w_qkv, w_out, b_out, idxs):
    """Scatter per-core live-row outputs into the full [B, N, DM] result.

    parts[c] is core c's [NL, DM] output (rows beyond n_live are pad).
    Dead-query rows all equal mean_j(v_j) @ w_out + b_out, exact in f32.
    """
    x = np.asarray(x, np.float32)
    w_qkv = np.asarray(w_qkv, np.float32)
    w_out = np.asarray(w_out, np.float32)
    b_out = np.asarray(b_out, np.float32)
    w3 = w_qkv.reshape(DM, H, 3, DH)
    wv_full = np.ascontiguousarray(w3[:, :, 2, :].reshape(DM, H * DH))
    out = np.empty((B, N, DM), np.float32)
    for b in range(B):
        idx = idxs[b]
        nl = len(idx)
        live = parts[HG * b][:nl] + parts[HG * b + 1][:nl] + b_out[None, :]
        dead_row = (x[b].mean(axis=0) @ wv_full) @ w_out + b_out
        out[b] = dead_row[None, :]
        out[b][idx] = live
    return out


def kernel(x, w_qkv, w_out, b_out, mask):
    from concourse.bass_utils import run_bass_kernel_spmd

    in_maps, idxs, ntl = _host_prep(x, w_qkv, w_out, b_out, mask)
    key = ("nc", ntl)
    if key not in _CACHE:
        _CACHE[key] = _build_program(ntl)
    nc = _CACHE[key]

    res = run_bass_kernel_spmd(nc, in_maps, list(range(NCORES))).results
    parts = [res[c]["out"] for c in range(NCORES)]
    return _assemble(parts, x, w_qkv, w_out, b_out, idxs)
